# revision 29
# baseline (speedup 1.0000x reference)
"""Trainium2 Bass kernel for nn_LIFLayer (T=512, B=64, C_IN=C_OUT=512).

Data-parallel over batch (8 batches/core, no collectives), fused single pass.

Gate recurrence is kept directly in PSUM as a decayed running state:

  S_{t+1} = A'*S_t + [sig_t * slow_{t-1}] @ (B'*Ws^T) + H_t
  H_t     = G_{t+1} - A'*G_t + x_t@Ws^T          (precomputed per 16-tick chunk)
  G_t     = x_t@Wx^T + b
  slow_t  = A'*slow_{t-1} + B'*(sig_t*slow_{t-1}) + x_t

using the linearization d = 0.995^(0.9*sig+0.05) ~= A' + B'*sig (max abs err
~2.5e-6). The A' decay cannot be applied inside PSUM, so within a 16-tick
window the state is stored as S_t * A'^-tau: the chain stt scales its output
by kappa_tau = A'^-(tau+1), H rows are pre-scaled by a per-partition lambda,
and sigmoid reads PSUM with ACT scale A'^tau. At window boundaries the bank
is drained (ACT copy, scale A'^16) and re-seeded into a fresh bank.

Per-tick critical chain: 4 bf16 matmuls (stationary sig*slow carries the
full sigmoid signal, so bf16 is safe) -> sigmoid -> 4 PE transposes -> one
DVE stt. Everything else (H matmul, drain/seed, x staging, fast/cur/v
deferred work) is filler scheduled into per-tick engine-idle slots; xt and
slow live entirely in SBUF.
"""

import math
from collections import defaultdict
import numpy as np

T, B, C = 512, 64, 512
CO = 512
NCORES = 8
BL = B // NCORES  # 8 batches per core
ALPHA = 0.9
A_FAST = 0.9
A_SLOW = 0.995

# linearization of d = A_SLOW**(0.9*sig + 0.05) = AP0 + BP1*sig
_L = math.log(A_SLOW)
_c1 = 0.9 * _L
_M2 = math.exp(0.05 * _L) * math.exp(_c1 / 2.0)
BP1 = _M2 * _c1                  # B'
AP0 = _M2 * (1.0 - _c1 / 2.0)    # A'

_NC_CACHE = {}


def build_nc(t_steps=T):
    import concourse.bass as bass
    import concourse.bacc as bacc
    import concourse.mybir as mybir
    from concourse.tile import TileContext
    from contextlib import ExitStack

    f32 = mybir.dt.float32
    f32r = mybir.dt.float32r
    bf16 = mybir.dt.bfloat16
    AF = mybir.ActivationFunctionType
    OP = mybir.AluOpType

    NCH = t_steps // 16          # 16-tick windows (also deferred groups)

    nc = bacc.Bacc()

    seq_l = nc.dram_tensor("seq_l", [t_steps, BL, C], f32, kind="ExternalInput")
    wsT_d = nc.dram_tensor("wsT", [C, C], f32r, kind="ExternalInput")
    wxT_d = nc.dram_tensor("wxT", [C, C], f32r, kind="ExternalInput")
    wxTn_d = nc.dram_tensor("wxTn", [C, C], f32r, kind="ExternalInput")
    wsTB2_d = nc.dram_tensor("wsTB2", [C, C], bf16, kind="ExternalInput")
    w01_d = nc.dram_tensor("w01", [C, CO], f32r, kind="ExternalInput")
    bias_d = nc.dram_tensor("biasv", [1, C], f32r, kind="ExternalInput")
    hbias_d = nc.dram_tensor("hbias", [1, C], f32r, kind="ExternalInput")
    lam_d = nc.dram_tensor("lam", [128, 1], f32, kind="ExternalInput")
    eye8_d = nc.dram_tensor("eye8", [8, 8], f32r, kind="ExternalInput")
    eye8b_d = nc.dram_tensor("eye8b", [8, 8], bf16, kind="ExternalInput")
    eye128_d = nc.dram_tensor("eye128", [128, 128], f32, kind="ExternalInput")
    ones_d = nc.dram_tensor("ones1", [1, 128], f32r, kind="ExternalInput")
    out_d = nc.dram_tensor("out_l", [BL, CO], f32, kind="ExternalOutput")

    with TileContext(nc) as tc, ExitStack() as ctx:
        consts = ctx.enter_context(tc.tile_pool(name="consts", bufs=1))
        wsT_sb = consts.tile([128, 4, C], f32r)
        wxT_sb = consts.tile([128, 4, C], f32r)
        wxTn_sb = consts.tile([128, 4, C], f32r)
        wsTB2_sb = consts.tile([128, 4, C], bf16)
        w01_sb = consts.tile([128, 4, CO], f32r)
        bias_sb = consts.tile([1, C], f32r)
        hbias_sb = consts.tile([1, C], f32r)
        lam_sb = consts.tile([128, 1], f32)
        eye8_sb = consts.tile([8, 8], f32r)
        eye8b_sb = consts.tile([8, 8], bf16)
        eye128_sb = consts.tile([128, 128], f32)
        ones_sb = consts.tile([1, 128], f32r)
        c09 = consts.tile([128, 16], f32)
        zro1 = consts.tile([1, 8], f32r)

        nc.sync.dma_start(wsT_sb, wsT_d.rearrange("(k p) j -> p k j", p=128))
        nc.sync.dma_start(wxT_sb, wxT_d.rearrange("(k p) j -> p k j", p=128))
        nc.sync.dma_start(wxTn_sb, wxTn_d.rearrange("(k p) j -> p k j", p=128))
        nc.sync.dma_start(wsTB2_sb, wsTB2_d.rearrange("(k p) j -> p k j", p=128))
        nc.sync.dma_start(w01_sb, w01_d.rearrange("(k p) j -> p k j", p=128))
        nc.sync.dma_start(bias_sb, bias_d[:, :])
        nc.sync.dma_start(hbias_sb, hbias_d[:, :])
        nc.sync.dma_start(lam_sb, lam_d[:, :])
        nc.sync.dma_start(eye8_sb, eye8_d[:, :])
        nc.sync.dma_start(eye8b_sb, eye8b_d[:, :])
        nc.sync.dma_start(eye128_sb, eye128_d[:, :])
        nc.sync.dma_start(ones_sb, ones_d[:, :])
        nc.vector.memset(c09, A_FAST)
        nc.vector.memset(zro1.bitcast(f32), 0.0)

        # persistent state (full history, SBUF-resident)
        state = ctx.enter_context(tc.tile_pool(name="state", bufs=1))
        # x transposed channel-major, t-major: tick t stored at row t+1
        xt = state.tile([128, 4, t_steps + 2, BL], f32r)
        slow = state.tile([128, 4, t_steps, BL], f32r)
        v_st = state.tile([128, 4, 8], f32)
        acc = state.tile([128, 4, 8], f32)
        nc.vector.memset(xt[:, :, 0, :].bitcast(f32), 0.0)
        nc.vector.memset(xt[:, :, t_steps + 1, :].bitcast(f32), 0.0)
        nc.vector.memset(v_st, 0.0)
        nc.vector.memset(acc, 0.0)

        # rotating pools
        seqp = ctx.enter_context(tc.tile_pool(name="seqp", bufs=2))
        hsb = ctx.enter_context(tc.tile_pool(name="hsb", bufs=3))
        gtp = ctx.enter_context(tc.tile_pool(name="gtp", bufs=3))
        sigp = ctx.enter_context(tc.tile_pool(name="sigp", bufs=2))
        yp = ctx.enter_context(tc.tile_pool(name="yp", bufs=2))
        srp = ctx.enter_context(tc.tile_pool(name="srp", bufs=2))
        dpool = ctx.enter_context(tc.tile_pool(name="dpool", bufs=2))
        vpp = ctx.enter_context(tc.tile_pool(name="vpp", bufs=1))
        small = ctx.enter_context(tc.tile_pool(name="small", bufs=2))

        spsum = ctx.enter_context(tc.tile_pool(name="spsum", bufs=2, space="PSUM"))
        tpsum = ctx.enter_context(tc.tile_pool(name="tpsum", bufs=1, space="PSUM"))
        scrp = ctx.enter_context(tc.tile_pool(name="scrp", bufs=1, space="PSUM"))
        cpsum = ctx.enter_context(tc.tile_pool(name="cpsum", bufs=2, space="PSUM"))
        dmyp = ctx.enter_context(tc.tile_pool(name="dmyp", bufs=1, space="PSUM"))

        live = {
            "seqc": {}, "H_sb": {}, "H_ps": {}, "gt": {},
            "fast": {}, "fcar": {}, "vp64": {}, "cur": {}, "nsum": {},
        }

        # ---------------- scheduled emission closures -----------------------
        def em_seqc(u):
            t0 = u * 16
            seqc = seqp.tile([128, C], f32, tag="seqc")
            nc.sync.dma_start(
                seqc, seq_l[t0:t0 + 16].rearrange("t b c -> (t b) c")
            )
            live["seqc"][u] = seqc

        def em_xt(u):
            seqc = live["seqc"].pop(u)
            scr = scrp.tile([128, C], f32, tag="scr")
            xt_ps = scr.rearrange("p (k t b) -> p k t b", k=4, t=16)
            for k in range(4):
                nc.tensor.transpose(
                    xt_ps[:, k].rearrange("p t b -> p (t b)"),
                    seqc[:, k * 128:(k + 1) * 128],
                    eye128_sb,
                )
            nc.scalar.activation(
                xt[:, :, 16 * u + 1:16 * u + 17, :], xt_ps, AF.Copy,
            )

        def em_hmm(u, part):
            # H_t rows for ticks t = 16u..16u+15 (t-major partitions):
            #   H_t = x_t@wsT + x_{t+1}@wxT - A'*(x_t@wxT) + (1-A')*b
            # x_t lives at xt row t+1; two matmuls per part
            if part == 0:
                H_ps = scrp.tile([128, C], f32, tag="scr")
                live["H_ps"][u] = H_ps
            else:
                H_ps = live["H_ps"][u]
            for i in (0, 1):
                k = (2 * part + i) % 4
                wt, off = ((wsT_sb, 1), (wxT_sb, 2), (wxTn_sb, 1))[part // 2]
                nc.tensor.matmul(
                    H_ps,
                    xt[:, k, 16 * u + off:16 * u + off + 16, :].rearrange(
                        "p t b -> p (t b)"),
                    wt[:, k, :],
                    start=(part == 0 and i == 0), stop=False,
                )
            if part == 5:
                nc.tensor.matmul(
                    H_ps, ones_sb, hbias_sb, start=False, stop=True)

        def em_hcopy(u):
            H_ps = live["H_ps"].pop(u)
            H_sb = hsb.tile([128, C], bf16, tag="H_sb")
            # rows pre-scaled by lambda[p] = A'^-(p//8 + 1)
            nc.scalar.activation(H_sb, H_ps, AF.Copy, scale=lam_sb)
            live["H_sb"][u] = H_sb
            live["H_sb"].pop(u - 3, None)

        def em_gt(t):
            u, j = t // 16, t % 16
            H_sb = live["H_sb"][u]
            gt = gtp.tile([8, C], bf16, tag="gt")
            nc.sync.dma_start(gt, H_sb[8 * j:8 * j + 8, :])
            live["gt"][t] = gt

        # deferred per-chunk work -------------------------------------------
        def em_scan(w, i):
            if i == 0:
                fast = dpool.tile([128, 4, 16, BL], f32r, tag="fast")
                live["fast"][w] = fast
            fast = live["fast"][w]
            prev = live["fcar"].get(w - 1)
            for n in range(4 * i, 4 * i + 4):
                k, b = n // BL, n % BL
                nc.vector.tensor_tensor_scan(
                    fast[:, k, :, b],
                    c09,
                    xt[:, k, 16 * w + 1:16 * w + 17, b],
                    initial=(0.0 if w == 0 else prev[:, k, b:b + 1]),
                    op0=OP.mult, op1=OP.add,
                )
            if i == 7:
                live["fcar"].pop(w - 1, None)

        def em_zx(w, i):
            fast = live["fast"][w]
            if i == 0:
                fcar = small.tile([128, 4, 8], f32, tag="fcar")
                live["fcar"][w] = fcar
                nc.vector.tensor_scalar(
                    fcar, fast[:, :, 15, :], 1.0, None, op0=OP.mult,
                )
            for k in range(2 * i, 2 * i + 2):
                nc.vector.scalar_tensor_tensor(
                    fast[:, k],
                    xt[:, k, 16 * w + 1:16 * w + 17, :],
                    2.0,
                    fast[:, k],
                    op0=OP.mult, op1=OP.add,
                )

        def em_cur(w, i):
            # two 128-row matmuls per call (i in 0..15)
            if i == 0:
                cur = cpsum.tile([128, 4, 16, BL], f32, tag="cur")
                live["cur"][w] = cur
            cur = live["cur"][w]
            fast = live["fast"][w]
            for n in (2 * i, 2 * i + 1):
                m, k, src = n // 8, (n % 8) // 2, n % 2
                mov = (fast[:, k] if src == 0
                       else slow[:, k, 16 * w:16 * w + 16, :])
                nc.tensor.matmul(
                    cur[:, m].rearrange("p t b -> p (t b)"),
                    w01_sb[:, k, m * 128:(m + 1) * 128],
                    mov.rearrange("p t b -> p (t b)"),
                    start=(n % 8 == 0), stop=(n % 8 == 7),
                )

        def em_vstep(w, i):
            # one v tick per call (i in 0..15)
            if i == 0:
                vp64 = vpp.tile([128, 4, 16, BL], f32, tag="vp64")
                live["vp64"][w] = vp64
            vp64 = live["vp64"][w]
            cur = live["cur"][w]
            nc.vector.scalar_tensor_tensor(
                vp64[:, :, i, :], v_st, ALPHA, cur[:, :, i, :],
                op0=OP.mult, op1=OP.add,
            )
            nc.vector.scalar_tensor_tensor(
                v_st, vp64[:, :, i, :], 1.0, vp64[:, :, i, :],
                op0=OP.is_le, op1=OP.mult,
            )
            if i == 15:
                live["cur"].pop(w, None)

        def em_spike(w, i):
            if i < 2:
                fast = live["fast"][w]
                vp64 = live["vp64"][w]
                if i == 0:
                    nsum = small.tile([128, 4, 8], f32, tag="nsum")
                    live["nsum"][w] = nsum
                nsum = live["nsum"][w]
                for k in (2 * i, 2 * i + 1):
                    nc.vector.tensor_scalar(
                        fast[:, k].rearrange("p t b -> p (t b)"),
                        vp64[:, k].rearrange("p t b -> p (t b)"),
                        1.0, None, op0=OP.is_le,
                    )
                    nc.vector.tensor_reduce(
                        nsum[:, k],
                        fast[:, k].rearrange("p t b -> p b t"),
                        axis=mybir.AxisListType.X, op=OP.add,
                    )
            else:
                nsum = live["nsum"].pop(w)
                nc.vector.tensor_tensor(acc, acc, nsum, op=OP.add)
                live["vp64"].pop(w, None)
                live["fast"].pop(w, None)

        PE_COST = {"em_xt": 600, "em_hmm": 430, "em_cur": 430}

        # ---------------- build the schedule -------------------------------
        warm, sched = [], defaultdict(list)

        def at(t, fn, *a):
            if t < 0:
                warm.append((t, fn, a))
            else:
                sched[t].append((fn, a))

        for u in range(NCH):
            at(16 * (u - 3) + 0, em_seqc, u)
            at(16 * (u - 2) + 1, em_xt, u)
            for part in range(6):
                at(16 * (u - 1) + 3 + part, em_hmm, u, part)
            at(16 * (u - 1) + 9, em_hcopy, u)
        for t in range(t_steps):
            at(t - 2, em_gt, t)
        for w in range(NCH):
            base = 16 * w + 32
            for i in range(8):
                at(base + i, em_scan, w, i)
            at(base + 8, em_zx, w, 0)
            at(base + 9, em_zx, w, 1)
            for i in range(16):
                at(base + 12 + i, em_cur, w, i)
            for i in range(16):
                at(base + 28 + i, em_vstep, w, i)
            for i in range(3):
                at(base + 44 + i, em_spike, w, i)

        dmy = dmyp.tile([8, 256], f32)

        warm.sort(key=lambda x: x[0])
        for _, fn, a in warm:
            fn(*a)

        # seed window 0 with S_0 = G_0 = x_0@wxT + b, plus H_0
        P_cur = spsum.tile([8, C], f32, tag="P")
        for k in range(4):
            nc.tensor.matmul(
                P_cur,
                xt[:, k, 1:2, :].rearrange("p t b -> p (t b)"),
                wxT_sb[:, k, :],
                start=(k == 0), stop=False, skip_group_check=True,
            )
        nc.tensor.matmul(P_cur, ones_sb[:, 0:8], bias_sb,
                         start=False, stop=False, skip_group_check=True)
        nc.tensor.matmul(P_cur, eye8b_sb, live["gt"].pop(0),
                         start=False, stop=False, skip_group_check=True)

        # ---------------- main tick loop -----------------------------------
        for t in range(t_steps):
            u, tau = t // 16, t % 16
            if tau == 0 and u >= 1:
                # drain runs on DVE concurrently with the sigmoid (both only
                # read the bank)
                Sres = srp.tile([8, C], f32r, tag="Sres")
                nc.vector.tensor_scalar(
                    Sres, P_cur, AP0 ** 16, None, op0=OP.mult,
                )
            if t >= 1:
                # 1) sigmoid of S_t = A'^tau * P (at tau=0 the bank still
                #    holds the previous window's state, scale A'^16)
                sig = sigp.tile([8, C], f32, tag="sig")
                sc = AP0 ** tau if tau >= 1 else AP0 ** 16
                nc.scalar.activation(sig, P_cur, AF.Sigmoid, scale=sc)
            if tau == 0 and u >= 1:
                P_new = spsum.tile([8, C], f32, tag="P")
                nc.tensor.matmul(P_new, eye8_sb, Sres, start=True,
                                 stop=False, skip_group_check=True)
                nc.tensor.matmul(P_new, eye8b_sb, live["gt"].pop(t),
                                 start=False, stop=False,
                                 skip_group_check=True)
                P_cur = P_new
            # 3) PE/DVE fillers run during the sigmoid window; pad with
            #    side-effect-free ldweights to keep the PE clock ramped
            pe_ns = 0
            for fn, a in sched.pop(t, ()):
                fn(*a)
                pe_ns += PE_COST.get(fn.__name__, 0)
            for _ in range(max(0, (700 - pe_ns) // 110)):
                nc.tensor.matmul(dmy, zro1, bias_sb[:, 0:256],
                                 start=True, stop=True)
            if t >= 1:
                # 4) transposes to channel-major
                sigT = tpsum.tile([128, 4, 8], f32, tag="sigT")
                for k in range(4):
                    nc.tensor.transpose(
                        sigT[:, k, :], sig[:, k * 128:(k + 1) * 128],
                        eye128_sb[0:8, 0:8],
                    )
                # 5) this tick's H row into the S state (after sigma has
                #    read the bank; PSUM adds commute with the chain mms)
                if tau >= 1:
                    nc.tensor.matmul(P_cur, eye8b_sb, live["gt"].pop(t),
                                     start=False, stop=False,
                                     skip_group_check=True)
                # 6) chain stt: y = kappa * sig * slow_{t-1}
                y = yp.tile([128, 4, 8], bf16, tag="y")
                nc.vector.scalar_tensor_tensor(
                    y, sigT, AP0 ** (-(tau + 1)), slow[:, :, t - 1, :],
                    op0=OP.mult, op1=OP.mult,
                )
                # 7) chain matmuls into the S state
                for k in range(4):
                    nc.tensor.matmul(
                        P_cur, y[:, k], wsTB2_sb[:, k, :],
                        start=False, stop=False, skip_group_check=True,
                    )
                # 8) slow_t = A'*slow_{t-1} + (B'/kappa)*y + x_t
                r = small.tile([128, 4, 8], f32, tag="rr")
                nc.vector.scalar_tensor_tensor(
                    r, y, BP1 * AP0 ** (tau + 1), xt[:, :, t + 1, :],
                    op0=OP.mult, op1=OP.add,
                )
                nc.vector.scalar_tensor_tensor(
                    slow[:, :, t, :], slow[:, :, t - 1, :], AP0, r,
                    op0=OP.mult, op1=OP.add,
                )
            else:
                nc.vector.tensor_scalar(
                    slow[:, :, 0, :], xt[:, :, 1, :], 1.0, None, op0=OP.mult,
                )

        # tail: drain remaining scheduled work
        for t in sorted(sched.keys()):
            for fn, a in sched[t]:
                fn(*a)

        # ---------------- output -------------------------------------------
        res = state.tile([128, 4, 8], f32)
        nc.vector.tensor_scalar(
            res.rearrange("p m b -> p (m b)"),
            acc.rearrange("p m b -> p (m b)"),
            -1.0 / t_steps, 1.0,
            op0=OP.mult, op1=OP.add,
        )
        resT_flat = spsum.tile([8, C], f32, tag="P")
        resT_ps = resT_flat.rearrange("b (m p) -> b m p", m=4)
        for m in range(4):
            nc.tensor.transpose(resT_ps[:, m, :], res[:, m, :], eye128_sb)
        resT = state.tile([8, C], f32)
        nc.scalar.activation(resT, resT_flat, AF.Copy)
        nc.sync.dma_start(out_d[:, :], resT)

    nc.finalize()
    return nc


def _prep_shared(seq, W, ctrl_w, ctrl_b):
    import ml_dtypes
    f = np.float32
    wsT = np.ascontiguousarray(ctrl_w[:, C:].T, dtype=f)
    wxT = np.ascontiguousarray(ctrl_w[:, :C].T, dtype=f)
    wxTn = np.ascontiguousarray(-AP0 * wxT, dtype=f)
    wsTB2 = np.ascontiguousarray(BP1 * wsT).astype(ml_dtypes.bfloat16)
    w01 = np.ascontiguousarray((1.0 - ALPHA) * 0.5 * W, dtype=f)
    bias = np.ascontiguousarray(ctrl_b[None, :], dtype=f)
    hbias = np.ascontiguousarray((1.0 - AP0) * ctrl_b[None, :], dtype=f)
    lam = np.asarray(
        [AP0 ** (-(p // 8 + 1)) for p in range(128)], dtype=f
    ).reshape(128, 1)
    eye8 = np.eye(8, dtype=f)
    eye8b = np.eye(8).astype(ml_dtypes.bfloat16)
    eye128 = np.eye(128, dtype=f)
    ones1 = np.ones((1, 128), dtype=f)
    return dict(wsT=wsT, wxT=wxT, wxTn=wxTn, wsTB2=wsTB2, w01=w01,
                biasv=bias, hbias=hbias, lam=lam, eye8=eye8, eye8b=eye8b,
                eye128=eye128, ones1=ones1)


LAST_EXEC_NS = None


def kernel(seq, W, ctrl_w, ctrl_b):
    global LAST_EXEC_NS
    import os
    from concourse.bass_utils import run_bass_kernel_spmd

    seq = np.asarray(seq, dtype=np.float32)
    t_steps = seq.shape[0]
    if t_steps not in _NC_CACHE:
        _NC_CACHE[t_steps] = build_nc(t_steps)
    nc = _NC_CACHE[t_steps]

    shared = _prep_shared(seq, np.asarray(W), np.asarray(ctrl_w),
                          np.asarray(ctrl_b))
    in_maps = []
    for c in range(NCORES):
        m = dict(shared)
        m["seq_l"] = np.ascontiguousarray(seq[:, c * BL:(c + 1) * BL, :])
        in_maps.append(m)

    trace = bool(os.environ.get("KERNEL_TRACE"))
    results = run_bass_kernel_spmd(
        nc, in_maps, core_ids=list(range(NCORES)), trace=trace
    )
    LAST_EXEC_NS = results.exec_time_ns
    return np.concatenate([res["out_l"] for res in results.results], axis=0)


if __name__ == "__main__":
    import reference

    inputs = {k: np.asarray(v) for k, v in reference.setup_inputs().items()}
    out = kernel(**inputs)
    print("kernel output", out.shape, out.dtype, out.mean())
